# revision 68
# baseline (speedup 1.0000x reference)
"""Trainium2 Bass kernel for nn_NestedNarx: batched NARX MLP over basins.

Math (from the reference):
  For t >= 3:  xt(t) = delayed features of x  ->  h = relu(W_in xt + b_in)
               a = tanh(W_ih h + b_ih + b_hh)
               y(t) = W_out a + b_out
  For t < 3:   y(t) = x[t, :, 7]

The 32 reference features (duplicated t-1 slice) fold into 24 distinct
features = all 8 channels at delays 1,2,3.

v2 design (pair-granular, mixed precision):
  - basins processed in pairs (2 basins); per core 64 pairs x 8 time chunks.
  - L1 (bf16): K=49 (2 basins x 24 delay-stacked feats + ones row that
    carries b_in), M=128 (2 basins x 64 h), N=512.  Host prepends the
    delay stack in [quad, 49, 4 pairs, T] layout.
  - relu: DVE (mostly) / ACT (some, for engine balance), PSUM -> SBUF fp8.
  - L2 (fp8 DoubleRow): rhs = H duo tile [128, 2, 512] holding both pairs
    of a duo; k-tile j of the DR matmul reads pair j's half.  The even
    pair uses stationary variant (D2, 0), the odd pair (0, D2) -- the
    sibling half is read but annihilated by zero weights.  Cost N/2.
  - tanh (ACT, [128, 1024] wide) -> A duo fp8.
  - L3 (fp8 DoubleRow): same even/odd trick, M=2; each pair's matmul
    writes psY rows [2p, 2p+2) of a [128, 512] bank that accumulates all
    128 basins of the chunk.  One bias-add + one DMA per chunk.
  - PSUM: psH[128,512]x2 + psA[128,2,512]x2 + psY[128,512]x2 = 8 banks.

Numerics (host-simulated): bf16 L1 + fp8e4m3 L2/L3 with RNE plus mean
bias corrections gives max|err| ~= 0.071 vs max|y| 4.2 (rel ~1.7e-2).
"""

import os
import sys

import numpy as np

for _p in ("/opt/trn_rl_repo",):
    if _p not in sys.path and os.path.isdir(_p):
        sys.path.insert(0, _p)

import ml_dtypes

import concourse.bass as bass
import concourse.mybir as mybir
from concourse.tile import TileContext




F32 = mybir.dt.float32
BF16 = mybir.dt.bfloat16
F8 = mybir.dt.float8e4
AF = mybir.ActivationFunctionType
DR = mybir.MatmulPerfMode.DoubleRow

NP_F8 = mybir.dt.np(F8)
NP_BF16 = mybir.dt.np(BF16)

T = 4096
NG_ALL = 1024
NCORES = 8
G_CORE = NG_ALL // NCORES  # 128 basins per core
NQUAD = 16                 # quads of 4 pairs (8 basins)
HID = 64
CH = 512                   # time chunk (PSUM bank width in fp32)
NCHUNK = T // CH
KS = 49                    # 2 basins x 24 stacked features + ones row


def _split_multiwaits(nc):
    """This container's walrus accepts only ONE sem wait per instruction
    ("Too many sync wait commands").  Hoist surplus waits onto single-wait
    NoOps inserted immediately before the instruction on the same engine
    queue (engines execute block-order, so gating is preserved)."""
    uid = [0]
    for fn in nc.m.functions:
        for bb in fn.blocks:
            new = []
            for inst in bb.instructions:
                si = inst.sync_info
                waits = list(si.on_wait) if si is not None and si.on_wait else []
                if len(waits) > 1:
                    for w in waits[:-1]:
                        uid[0] += 1
                        new.append(
                            mybir.InstNoOp(
                                name=f"{inst.name}-sw{uid[0]}",
                                engine=inst.engine,
                                bass_nofuse=True,
                                sync_info=mybir.SyncInfo(on_wait=[w], on_update=[]),
                            )
                        )
                    si.on_wait = waits[-1:]
                new.append(inst)
            bb.instructions = new
    return nc


def _dedupe_ldweights(nc):
    """Matmuls whose stationary operand equals the previous PE matmul's can
    skip the implicit LDWEIGHTS (walrus honors InstMatmult.ldweights=False
    for non-fp32 dtypes), removing the weight reload from the PE critical
    path."""
    n = 0
    for fn in nc.m.functions:
        for bb in fn.blocks:
            last = None
            for inst in bb.instructions:
                if inst.engine != mybir.EngineType.PE:
                    continue
                if not isinstance(inst, mybir.InstMatmult):
                    continue
                w = inst.ins[1]
                key = (
                    str(w.memref),
                    w.offset,
                    tuple(tuple(d) for d in w.ap),
                    inst.perf_mode,
                    inst.is_transpose,
                )
                if key == last:
                    inst.ldweights = False
                    n += 1
                last = key
    return n


def build_nc(split_waits=True, act_relu_mod=6):
    nc = bass.Bass()
    xp = nc.declare_dram_parameter("xp", [NQUAD, KS, 4, T], BF16, isOutput=False)
    l1 = nc.declare_dram_parameter("l1", [KS, 128], BF16, isOutput=False)
    l2 = nc.declare_dram_parameter("l2", [128, 2, 2, 128], F8, isOutput=False)
    # l3 dims: [K=128, j k-tile (= duo pair), i2 duo-slot in stripe, M=32]
    l3 = nc.declare_dram_parameter("l3", [128, 2, 8, 32], F8, isOutput=False)
    b2 = nc.declare_dram_parameter("b2", [128, 1], F32, isOutput=False)
    bo = nc.declare_dram_parameter("bo", [128, 1], F32, isOutput=False)
    y = nc.declare_dram_parameter("y", [128, T], F32, isOutput=True)

    with TileContext(nc) as tc:
        with (
            tc.tile_pool(name="const", bufs=1) as constp,
            tc.tile_pool(name="xs", bufs=4) as xsp,
            tc.tile_pool(name="hpool", bufs=7) as hpl,
            tc.tile_pool(name="apool", bufs=7) as apl,
            tc.tile_pool(name="yout", bufs=2) as youtp,
            tc.tile_pool(name="psumh", bufs=3, space=bass.MemorySpace.PSUM) as psumhp,
            tc.tile_pool(name="psuma", bufs=2, space=bass.MemorySpace.PSUM) as psumap,
            tc.tile_pool(name="psumy", bufs=1, space=bass.MemorySpace.PSUM) as psumyp,
        ):
            # ---- constants (loaded once) ----
            l1t = constp.tile([KS, 128], BF16, name="l1t")
            nc.sync.dma_start(out=l1t, in_=l1[:])
            l2t = constp.tile([128, 2, 2, 128], F8, name="l2t")
            nc.sync.dma_start(out=l2t, in_=l2[:])
            l3t = constp.tile([128, 2, 8, 32], F8, name="l3t")
            nc.sync.dma_start(out=l3t, in_=l3[:])
            b2t = constp.tile([128, 1], F32, name="b2t")
            nc.sync.dma_start(out=b2t, in_=b2[:])
            bot = constp.tile([128, 1], F32, name="bot")
            nc.sync.dma_start(out=bot, in_=bo[:])

            # ---- PE warm-up ----
            # the first real matmuls otherwise run at the cold p-state (0.65
            # -> 1.2 GHz) while waiting on xs DMAs; a burst of dummy matmuls
            # on the already-loaded l1 constant ramps the clock in that window
            for _ in range(10):
                wt = psumhp.tile([128, 128], F32, name="warm", tag="psH")
                nc.tensor.matmul(wt, l1t[:], l1t[:], start=True, stop=True)

            # ---- main loops: 3-stage software pipeline over duos ----
            # slot s: gen(s) = xs prefetch + L1 + relu; mid(s-1) = L2 + tanh;
            # out(s-2) = L3 (+ per-stripe bias/store).  The skew keeps the PE
            # queue free of waits on the vector engines.
            NDUO = NCHUNK * 32
            xs_tiles = {}
            Hd_t = {}
            Ad_t = {}
            psY_t = [None]

            def ensure_xs(gq):
                if gq in xs_tiles or gq >= NCHUNK * NQUAD:
                    return
                ck, qd = gq // NQUAD, gq % NQUAD
                xs = xsp.tile([KS, 4, CH], BF16, name="xs", tag="xs")
                nc.sync.dma_start(
                    out=xs, in_=xp[qd][:, :, ck * CH : ck * CH + CH]
                )
                xs_tiles[gq] = xs

            def gen(s, jjs=(0, 1)):
                gq = s // 2
                ensure_xs(gq)
                ensure_xs(gq + 1)
                ensure_xs(gq + 2)
                dd = s % 2
                if s not in Hd_t:
                    Hd_t[s] = hpl.tile([128, 2, CH], F8, name="Hd", tag="Hd")
                Hd = Hd_t[s]
                for jj in jjs:
                    pp = 2 * dd + jj
                    p = 2 * (s % 32) + jj
                    psH = psumhp.tile([128, CH], F32, name="psH", tag="psH")
                    nc.tensor.matmul(
                        psH, l1t, xs_tiles[gq][:, pp, :], start=True, stop=True
                    )
                    if p % act_relu_mod == act_relu_mod - 1:
                        # some relus ride the scalar engine for balance
                        nc.scalar.activation(Hd[:, jj, :], psH, AF.Relu)
                    else:
                        nc.vector.tensor_scalar_max(Hd[:, jj, :], psH, 0.0)
                if s % 2 == 1 and 1 in jjs:
                    xs_tiles.pop(gq, None)

            psA_t = {}

            def _rev_ktiles(a):
                # reversed k-tile view [128, 2, CH]: j=0 -> odd half, j=1 ->
                # even half, so the odd pair's L2 can reuse the even
                # stationary (weights live in k-tile 0 = the pair itself)
                import bass_rust

                r = a.copy()
                ap = [list(d) for d in r.ap]
                ap[1] = [-ap[1][0], ap[1][1]]
                r.ap = bass_rust.VecI64Pair(ap)
                r.offset = r.offset + CH
                return r

            def mid2(s):
                # all four L2s of duos s, s+1 share one stationary
                psA_t[s] = psumap.tile([128, 2, CH], F32, name="psA", tag="psA")
                psA_t[s + 1] = psumap.tile([128, 2, CH], F32, name="psA", tag="psA")
                # duo-major order: duo s+1's psA buffer (freed by the most
                # recent tanh) isn't touched until two matmuls in
                for ss in (s, s + 1):
                    for jj in range(2):
                        rhs = Hd_t[ss][:]
                        if jj == 1:
                            rhs = _rev_ktiles(rhs)
                        nc.tensor.matmul(
                            psA_t[ss][:, jj, :],
                            l2t[:, :, 0, :],
                            rhs,
                            start=True,
                            stop=True,
                            perf_mode=DR,
                        )

            def mid2_tail(s):
                for ss in (s, s + 1):
                    Hd_t.pop(ss)
                    psA = psA_t.pop(ss)
                    Ad = apl.tile([128, 2, CH], F8, name="Ad", tag="Ad")
                    nc.scalar.activation(Ad, psA, AF.Tanh, bias=b2t)
                    Ad_t[ss] = Ad

            def out(s):
                ck = s // 32
                Ad = Ad_t.pop(s)
                # one DR matmul covers both pairs of the duo: k-tile j = pair j,
                # weights target columns 4*i2 + 2*j + {0,1}
                i2 = (s % 32) % 8
                if i2 == 0:
                    psY_t[0] = psumyp.tile([32, CH], F32, name="psY", tag="psY")
                nc.tensor.matmul(
                    psY_t[0],
                    l3t[:, :, i2, :],
                    Ad[:],
                    start=(i2 == 0),
                    stop=(i2 == 7),
                    perf_mode=DR,
                )
                if s % 8 == 7:
                    S = (s % 32) // 8
                    ysb = youtp.tile([32, CH], F32, name="ysb", tag="ysb")
                    nc.vector.tensor_scalar_add(ysb, psY_t[0], bot[0:32])
                    nc.sync.dma_start(
                        out=y[32 * S : 32 * S + 32, ck * CH : ck * CH + CH],
                        in_=ysb,
                    )

            for u in range(0, NDUO + 6, 2):
                if 0 <= u - 4 < NDUO:
                    out(u - 4)
                    out(u - 3)
                if u < NDUO:
                    gen(u)
                if 0 <= u - 2 < NDUO:
                    mid2(u - 2)
                if u < NDUO:
                    gen(u + 1)
                if 0 <= u - 2 < NDUO:
                    mid2_tail(u - 2)
    _dedupe_ldweights(nc)
    if split_waits:
        _split_multiwaits(nc)
    return nc


def _q8(a):
    return np.asarray(a, NP_F8).astype(np.float32)


def prep_weights(W_in, b_in, W_ih, b_ih, b_hh, W_out, b_out, x):
    W_in = np.asarray(W_in, np.float32)
    W_ih = np.asarray(W_ih, np.float32)
    w = np.asarray(W_out, np.float32)[0]
    b_in = np.asarray(b_in, np.float32)
    bb = np.asarray(b_ih, np.float32) + np.asarray(b_hh, np.float32)
    bout = float(np.asarray(b_out, np.float32)[0])

    # fold the 32 reference features (duplicated t-1 slice) into 24:
    # A[d-1] [64, 8] acts on all 8 channels of x[t-d]
    A = np.zeros((3, HID, 8), np.float32)
    A[0, :, 0:7] = W_in[:, 0:7] + W_in[:, 21:28]
    A[0, :, 7] = W_in[:, 28] + W_in[:, 31]
    A[1, :, 0:7] = W_in[:, 14:21]
    A[1, :, 7] = W_in[:, 30]
    A[2, :, 0:7] = W_in[:, 7:14]
    A[2, :, 7] = W_in[:, 29]

    # l1 [49, 128]: row 24j + 8(d-1) + c -> col 64j + m: A[d-1][m, c];
    # ones row 48 carries b_in.
    l1 = np.zeros((KS, 128), np.float32)
    for j in range(2):
        for d in range(3):
            l1[24 * j + 8 * d : 24 * j + 8 * d + 8, 64 * j : 64 * j + 64] = A[d].T
        l1[48, 64 * j : 64 * j + 64] = b_in
    l1 = l1.astype(NP_BF16)

    # l2 [128, (j k-tile), (v parity variant), 128] fp8: D2 = diag(W2^T, W2^T)
    D2 = np.zeros((128, 128), np.float32)
    D2[0:64, 0:64] = W_ih.T
    D2[64:128, 64:128] = W_ih.T
    D2q = _q8(D2)
    l2 = np.zeros((128, 2, 2, 128), np.float32)
    l2[:, 0, 0, :] = D2q
    l2[:, 1, 1, :] = D2q
    l2 = l2.astype(NP_F8)

    # l3 [128, (j k-tile = duo pair), (i2 duo slot), 32] fp8:
    # k-tile j, slot i2 -> col 4*i2 + 2*j (+1 for odd basin rows 64:128)
    wq = _q8(w)
    l3 = np.zeros((128, 2, 8, 32), np.float32)
    for i2 in range(8):
        for j in range(2):
            l3[0:64, j, i2, 4 * i2 + 2 * j] = wq
            l3[64:128, j, i2, 4 * i2 + 2 * j + 1] = wq
    l3 = l3.astype(NP_F8)

    # Mean bias corrections for the fp8 quantization of W2/h and w/a,
    # estimated from a data slice.
    xs = np.asarray(x[:512, :256, :], np.float32)
    xq = xs.astype(NP_BF16).astype(np.float32)
    z1 = np.zeros((512, 256, HID), np.float32)
    Aq = np.asarray(A, NP_BF16).astype(np.float32)
    for d in (1, 2, 3):
        z1[d:] += np.einsum("tgc,hc->tgh", xq[: 512 - d], Aq[d - 1], optimize=True)
    h = np.maximum(z1 + b_in, 0.0)
    h8 = _q8(h)
    hbar = h.reshape(-1, HID).mean(0)
    h8bar = h8.reshape(-1, HID).mean(0)
    W2q = _q8(W_ih)
    db2 = W_ih @ hbar - W2q @ h8bar
    z2 = np.einsum("tgh,oh->tgo", h8, W2q, optimize=True) + bb + db2
    a = np.tanh(z2)
    a8 = _q8(a)
    dbo = w @ a.reshape(-1, HID).mean(0) - _q8(w) @ a8.reshape(-1, HID).mean(0)

    b2v = np.concatenate([bb + db2, bb + db2]).astype(np.float32).reshape(128, 1)
    bov = np.full((128, 1), np.float32(bout + dbo))
    return l1, l2, l3, b2v, bov


def prep_x_core(x, core):
    """x [4096, 1024, 8] f32 -> xp [16, 49, 4, 4096] bf16 delay-stacked.

    basin_local = 8*quad + 2*pp + j; row f = 24*j + 8*(d-1) + c holds
    x[t-d, basin, c]; row 48 = 1.0 (bias row)."""
    xc = np.asarray(x[:, core * G_CORE : (core + 1) * G_CORE, :], np.float32)
    xcm = np.ascontiguousarray(xc.transpose(1, 2, 0))  # [128, 8, T]
    st = np.zeros((G_CORE, 3, 8, T), np.float32)
    for d in (1, 2, 3):
        st[:, d - 1, :, d:] = xcm[:, :, : T - d]
    # [128 = (quad, pp, j), 24, T] -> [16, 4, 2, 24, T] -> [16, 2, 24, 4, T]
    st = st.reshape(NQUAD, 4, 2, 24, T).transpose(0, 2, 3, 1, 4)
    out = np.empty((NQUAD, KS, 4, T), np.float32)
    out[:, :48] = st.reshape(NQUAD, 48, 4, T)
    out[:, 48] = 1.0
    return out.astype(NP_BF16)


_NC_CACHE = {}


def _get_nc():
    if "nc" not in _NC_CACHE:
        _NC_CACHE["nc"] = build_nc()
    return _NC_CACHE["nc"]


def kernel(x, W_in, b_in, W_ih, b_ih, W_hh, b_hh, W_out, b_out, _trace=False):
    from concourse.bass_utils import run_bass_kernel_spmd

    x = np.asarray(x, np.float32)
    l1, l2, l3, b2v, bov = prep_weights(
        W_in, b_in, W_ih, b_ih, b_hh, W_out, b_out, x
    )
    in_maps = []
    for core in range(NCORES):
        in_maps.append(
            {
                "xp": prep_x_core(x, core),
                "l1": l1,
                "l2": l2,
                "l3": l3,
                "b2": b2v,
                "bo": bov,
            }
        )
    nc = _get_nc()
    res = run_bass_kernel_spmd(nc, in_maps, list(range(NCORES)), trace=_trace)
    _NC_CACHE["last_result"] = res

    out = np.empty((T, NG_ALL, 1), np.float32)
    out[:3, :, 0] = x[:3, :, 7]
    for core in range(NCORES):
        yc = res.results[core]["y"]  # [128, T]
        out[3:, core * G_CORE : (core + 1) * G_CORE, 0] = yc[:, 3:].T
    return out


# revision 70
# speedup vs baseline: 1.2164x; 1.2164x over previous
"""Trainium2 Bass kernel for nn_NestedNarx: batched NARX MLP over basins.

Math (from the reference):
  For t >= 3:  xt(t) = delayed features of x  ->  h = relu(W_in xt + b_in)
               a = tanh(W_ih h + b_ih + b_hh)
               y(t) = W_out a + b_out
  For t < 3:   y(t) = x[t, :, 7]

The 32 reference features (duplicated t-1 slice) fold into 24 distinct
features = all 8 channels at delays 1,2,3.

v2 design (pair-granular, mixed precision):
  - basins processed in pairs (2 basins); per core 64 pairs x 8 time chunks.
  - L1 (bf16): K=49 (2 basins x 24 delay-stacked feats + ones row that
    carries b_in), M=128 (2 basins x 64 h), N=512.  Host prepends the
    delay stack in [quad, 49, 4 pairs, T] layout.
  - relu: DVE (mostly) / ACT (some, for engine balance), PSUM -> SBUF fp8.
  - L2 (fp8 DoubleRow): rhs = H duo tile [128, 2, 512] holding both pairs
    of a duo; k-tile j of the DR matmul reads pair j's half.  The even
    pair uses stationary variant (D2, 0), the odd pair (0, D2) -- the
    sibling half is read but annihilated by zero weights.  Cost N/2.
  - tanh (ACT, [128, 1024] wide) -> A duo fp8.
  - L3 (fp8 DoubleRow): same even/odd trick, M=2; each pair's matmul
    writes psY rows [2p, 2p+2) of a [128, 512] bank that accumulates all
    128 basins of the chunk.  One bias-add + one DMA per chunk.
  - PSUM: psH[128,512]x2 + psA[128,2,512]x2 + psY[128,512]x2 = 8 banks.

Numerics (host-simulated): bf16 L1 + fp8e4m3 L2/L3 with RNE plus mean
bias corrections gives max|err| ~= 0.071 vs max|y| 4.2 (rel ~1.7e-2).
"""

import os
import sys

import numpy as np

for _p in ("/opt/trn_rl_repo",):
    if _p not in sys.path and os.path.isdir(_p):
        sys.path.insert(0, _p)

import ml_dtypes

import concourse.bass as bass
import concourse.mybir as mybir
from concourse.tile import TileContext




F32 = mybir.dt.float32
BF16 = mybir.dt.bfloat16
F8 = mybir.dt.float8e4
AF = mybir.ActivationFunctionType
DR = mybir.MatmulPerfMode.DoubleRow

NP_F8 = mybir.dt.np(F8)
NP_BF16 = mybir.dt.np(BF16)

T = 4096
NG_ALL = 1024
NCORES = 8
G_CORE = NG_ALL // NCORES  # 128 basins per core
NQUAD = 16                 # quads of 4 pairs (8 basins)
HID = 64
CH = 512                   # time chunk (PSUM bank width in fp32)
NCHUNK = T // CH
KS = 49                    # 2 basins x 24 stacked features + ones row


def _split_multiwaits(nc):
    """This container's walrus accepts only ONE sem wait per instruction
    ("Too many sync wait commands").  Hoist surplus waits onto single-wait
    NoOps inserted immediately before the instruction on the same engine
    queue (engines execute block-order, so gating is preserved)."""
    uid = [0]
    for fn in nc.m.functions:
        for bb in fn.blocks:
            new = []
            for inst in bb.instructions:
                si = inst.sync_info
                waits = list(si.on_wait) if si is not None and si.on_wait else []
                if len(waits) > 1:
                    for w in waits[:-1]:
                        uid[0] += 1
                        new.append(
                            mybir.InstNoOp(
                                name=f"{inst.name}-sw{uid[0]}",
                                engine=inst.engine,
                                bass_nofuse=True,
                                sync_info=mybir.SyncInfo(on_wait=[w], on_update=[]),
                            )
                        )
                    si.on_wait = waits[-1:]
                new.append(inst)
            bb.instructions = new
    return nc


def _dedupe_ldweights(nc):
    """Matmuls whose stationary operand equals the previous PE matmul's can
    skip the implicit LDWEIGHTS (walrus honors InstMatmult.ldweights=False
    for non-fp32 dtypes), removing the weight reload from the PE critical
    path."""
    n = 0
    for fn in nc.m.functions:
        for bb in fn.blocks:
            last = None
            for inst in bb.instructions:
                if inst.engine != mybir.EngineType.PE:
                    continue
                if not isinstance(inst, mybir.InstMatmult):
                    continue
                w = inst.ins[1]
                key = (
                    str(w.memref),
                    w.offset,
                    tuple(tuple(d) for d in w.ap),
                    inst.perf_mode,
                    inst.is_transpose,
                )
                if key == last:
                    inst.ldweights = False
                    n += 1
                last = key
    return n


def build_nc(split_waits=True, act_relu_mod=6):
    nc = bass.Bass()
    xp = nc.declare_dram_parameter("xp", [NQUAD, KS, 4, T], BF16, isOutput=False)
    l1 = nc.declare_dram_parameter("l1", [KS, 128], BF16, isOutput=False)
    l2 = nc.declare_dram_parameter("l2", [128, 2, 2, 128], F8, isOutput=False)
    # l3 dims: [K=128, j k-tile (= duo pair), i2 duo-slot in stripe, M=32]
    l3 = nc.declare_dram_parameter("l3", [128, 2, 8, 32], F8, isOutput=False)
    b2 = nc.declare_dram_parameter("b2", [128, 1], F32, isOutput=False)
    bo = nc.declare_dram_parameter("bo", [128, 1], F32, isOutput=False)
    y = nc.declare_dram_parameter("y", [128, T], F32, isOutput=True)

    with TileContext(nc) as tc:
        with (
            tc.tile_pool(name="const", bufs=1) as constp,
            tc.tile_pool(name="xs", bufs=4) as xsp,
            tc.tile_pool(name="hpool", bufs=7) as hpl,
            tc.tile_pool(name="apool", bufs=7) as apl,
            tc.tile_pool(name="yout", bufs=2) as youtp,
            tc.tile_pool(name="psumh", bufs=3, space=bass.MemorySpace.PSUM) as psumhp,
            tc.tile_pool(name="psuma", bufs=2, space=bass.MemorySpace.PSUM) as psumap,
            tc.tile_pool(name="psumy", bufs=1, space=bass.MemorySpace.PSUM) as psumyp,
        ):
            NDUO = NCHUNK * 32
            xs_tiles = {}
            Hd_t = {}
            Ad_t = {}
            psY_t = [None]

            def ensure_xs(gq):
                if gq in xs_tiles or gq >= NCHUNK * NQUAD:
                    return
                ck, qd = gq // NQUAD, gq % NQUAD
                xs = xsp.tile([KS, 4, CH], BF16, name="xs", tag="xs")
                nc.sync.dma_start(
                    out=xs, in_=xp[qd][:, :, ck * CH : ck * CH + CH]
                )
                xs_tiles[gq] = xs

            # ---- constants (loaded once); the first xs loads are issued
            # right after the small l1 weights so the critical path to the
            # first L1 matmul isn't serialized behind all five const DMAs
            l1t = constp.tile([KS, 128], BF16, name="l1t")
            nc.sync.dma_start(out=l1t, in_=l1[:])
            ensure_xs(0)
            ensure_xs(1)
            l2t = constp.tile([128, 2, 2, 128], F8, name="l2t")
            nc.sync.dma_start(out=l2t, in_=l2[:])
            l3t = constp.tile([128, 2, 8, 32], F8, name="l3t")
            nc.sync.dma_start(out=l3t, in_=l3[:])
            b2t = constp.tile([128, 1], F32, name="b2t")
            nc.sync.dma_start(out=b2t, in_=b2[:])
            bot = constp.tile([128, 1], F32, name="bot")
            nc.sync.dma_start(out=bot, in_=bo[:])

            # ---- main loops: 3-stage software pipeline over duos ----
            # slot s: gen(s) = xs prefetch + L1 + relu; mid(s-1) = L2 + tanh;
            # out(s-2) = L3 (+ per-stripe bias/store).  The skew keeps the PE
            # queue free of waits on the vector engines.

            def gen(s, jjs=(0, 1)):
                gq = s // 2
                ensure_xs(gq)
                ensure_xs(gq + 1)
                ensure_xs(gq + 2)
                dd = s % 2
                if s not in Hd_t:
                    Hd_t[s] = hpl.tile([128, 2, CH], F8, name="Hd", tag="Hd")
                Hd = Hd_t[s]
                for jj in jjs:
                    pp = 2 * dd + jj
                    p = 2 * (s % 32) + jj
                    psH = psumhp.tile([128, CH], F32, name="psH", tag="psH")
                    nc.tensor.matmul(
                        psH, l1t, xs_tiles[gq][:, pp, :], start=True, stop=True
                    )
                    if p % act_relu_mod == act_relu_mod - 1:
                        # some relus ride the scalar engine for balance
                        nc.scalar.activation(Hd[:, jj, :], psH, AF.Relu)
                    else:
                        nc.vector.tensor_scalar_max(Hd[:, jj, :], psH, 0.0)
                if s % 2 == 1 and 1 in jjs:
                    xs_tiles.pop(gq, None)

            psA_t = {}

            def _rev_ktiles(a):
                # reversed k-tile view [128, 2, CH]: j=0 -> odd half, j=1 ->
                # even half, so the odd pair's L2 can reuse the even
                # stationary (weights live in k-tile 0 = the pair itself)
                import bass_rust

                r = a.copy()
                ap = [list(d) for d in r.ap]
                ap[1] = [-ap[1][0], ap[1][1]]
                r.ap = bass_rust.VecI64Pair(ap)
                r.offset = r.offset + CH
                return r

            def mid2(s):
                # all four L2s of duos s, s+1 share one stationary
                psA_t[s] = psumap.tile([128, 2, CH], F32, name="psA", tag="psA")
                psA_t[s + 1] = psumap.tile([128, 2, CH], F32, name="psA", tag="psA")
                # duo-major order: duo s+1's psA buffer (freed by the most
                # recent tanh) isn't touched until two matmuls in
                for ss in (s, s + 1):
                    for jj in range(2):
                        rhs = Hd_t[ss][:]
                        if jj == 1:
                            rhs = _rev_ktiles(rhs)
                        nc.tensor.matmul(
                            psA_t[ss][:, jj, :],
                            l2t[:, :, 0, :],
                            rhs,
                            start=True,
                            stop=True,
                            perf_mode=DR,
                        )

            def mid2_tail(s):
                for ss in (s, s + 1):
                    Hd_t.pop(ss)
                    psA = psA_t.pop(ss)
                    Ad = apl.tile([128, 2, CH], F8, name="Ad", tag="Ad")
                    nc.scalar.activation(Ad, psA, AF.Tanh, bias=b2t)
                    Ad_t[ss] = Ad

            def out(s):
                ck = s // 32
                Ad = Ad_t.pop(s)
                # one DR matmul covers both pairs of the duo: k-tile j = pair j,
                # weights target columns 4*i2 + 2*j + {0,1}
                i2 = (s % 32) % 8
                if i2 == 0:
                    psY_t[0] = psumyp.tile([32, CH], F32, name="psY", tag="psY")
                nc.tensor.matmul(
                    psY_t[0],
                    l3t[:, :, i2, :],
                    Ad[:],
                    start=(i2 == 0),
                    stop=(i2 == 7),
                    perf_mode=DR,
                )
                if s % 8 == 7:
                    S = (s % 32) // 8
                    ysb = youtp.tile([32, CH], F32, name="ysb", tag="ysb")
                    nc.vector.tensor_scalar_add(ysb, psY_t[0], bot[0:32])
                    nc.sync.dma_start(
                        out=y[32 * S : 32 * S + 32, ck * CH : ck * CH + CH],
                        in_=ysb,
                    )

            for u in range(0, NDUO + 6, 2):
                if 0 <= u - 4 < NDUO:
                    out(u - 4)
                    out(u - 3)
                if u < NDUO:
                    gen(u)
                if 0 <= u - 2 < NDUO:
                    mid2(u - 2)
                if u < NDUO:
                    gen(u + 1)
                if 0 <= u - 2 < NDUO:
                    mid2_tail(u - 2)
    _dedupe_ldweights(nc)
    if split_waits:
        _split_multiwaits(nc)
    return nc


def _q8(a):
    return np.asarray(a, NP_F8).astype(np.float32)


def prep_weights(W_in, b_in, W_ih, b_ih, b_hh, W_out, b_out, x):
    W_in = np.asarray(W_in, np.float32)
    W_ih = np.asarray(W_ih, np.float32)
    w = np.asarray(W_out, np.float32)[0]
    b_in = np.asarray(b_in, np.float32)
    bb = np.asarray(b_ih, np.float32) + np.asarray(b_hh, np.float32)
    bout = float(np.asarray(b_out, np.float32)[0])

    # fold the 32 reference features (duplicated t-1 slice) into 24:
    # A[d-1] [64, 8] acts on all 8 channels of x[t-d]
    A = np.zeros((3, HID, 8), np.float32)
    A[0, :, 0:7] = W_in[:, 0:7] + W_in[:, 21:28]
    A[0, :, 7] = W_in[:, 28] + W_in[:, 31]
    A[1, :, 0:7] = W_in[:, 14:21]
    A[1, :, 7] = W_in[:, 30]
    A[2, :, 0:7] = W_in[:, 7:14]
    A[2, :, 7] = W_in[:, 29]

    # l1 [49, 128]: row 24j + 8(d-1) + c -> col 64j + m: A[d-1][m, c];
    # ones row 48 carries b_in.
    l1 = np.zeros((KS, 128), np.float32)
    for j in range(2):
        for d in range(3):
            l1[24 * j + 8 * d : 24 * j + 8 * d + 8, 64 * j : 64 * j + 64] = A[d].T
        l1[48, 64 * j : 64 * j + 64] = b_in
    l1 = l1.astype(NP_BF16)

    # l2 [128, (j k-tile), (v parity variant), 128] fp8: D2 = diag(W2^T, W2^T)
    D2 = np.zeros((128, 128), np.float32)
    D2[0:64, 0:64] = W_ih.T
    D2[64:128, 64:128] = W_ih.T
    D2q = _q8(D2)
    l2 = np.zeros((128, 2, 2, 128), np.float32)
    l2[:, 0, 0, :] = D2q
    l2[:, 1, 1, :] = D2q
    l2 = l2.astype(NP_F8)

    # l3 [128, (j k-tile = duo pair), (i2 duo slot), 32] fp8:
    # k-tile j, slot i2 -> col 4*i2 + 2*j (+1 for odd basin rows 64:128)
    wq = _q8(w)
    l3 = np.zeros((128, 2, 8, 32), np.float32)
    for i2 in range(8):
        for j in range(2):
            l3[0:64, j, i2, 4 * i2 + 2 * j] = wq
            l3[64:128, j, i2, 4 * i2 + 2 * j + 1] = wq
    l3 = l3.astype(NP_F8)

    # Mean bias corrections for the fp8 quantization of W2/h and w/a,
    # estimated from a data slice.
    xs = np.asarray(x[:512, :256, :], np.float32)
    xq = xs.astype(NP_BF16).astype(np.float32)
    z1 = np.zeros((512, 256, HID), np.float32)
    Aq = np.asarray(A, NP_BF16).astype(np.float32)
    for d in (1, 2, 3):
        z1[d:] += np.einsum("tgc,hc->tgh", xq[: 512 - d], Aq[d - 1], optimize=True)
    h = np.maximum(z1 + b_in, 0.0)
    h8 = _q8(h)
    hbar = h.reshape(-1, HID).mean(0)
    h8bar = h8.reshape(-1, HID).mean(0)
    W2q = _q8(W_ih)
    db2 = W_ih @ hbar - W2q @ h8bar
    z2 = np.einsum("tgh,oh->tgo", h8, W2q, optimize=True) + bb + db2
    a = np.tanh(z2)
    a8 = _q8(a)
    dbo = w @ a.reshape(-1, HID).mean(0) - _q8(w) @ a8.reshape(-1, HID).mean(0)

    b2v = np.concatenate([bb + db2, bb + db2]).astype(np.float32).reshape(128, 1)
    bov = np.full((128, 1), np.float32(bout + dbo))
    return l1, l2, l3, b2v, bov


def prep_x_core(x, core):
    """x [4096, 1024, 8] f32 -> xp [16, 49, 4, 4096] bf16 delay-stacked.

    basin_local = 8*quad + 2*pp + j; row f = 24*j + 8*(d-1) + c holds
    x[t-d, basin, c]; row 48 = 1.0 (bias row)."""
    xc = np.asarray(x[:, core * G_CORE : (core + 1) * G_CORE, :], np.float32)
    xcm = np.ascontiguousarray(xc.transpose(1, 2, 0))  # [128, 8, T]
    st = np.zeros((G_CORE, 3, 8, T), np.float32)
    for d in (1, 2, 3):
        st[:, d - 1, :, d:] = xcm[:, :, : T - d]
    # [128 = (quad, pp, j), 24, T] -> [16, 4, 2, 24, T] -> [16, 2, 24, 4, T]
    st = st.reshape(NQUAD, 4, 2, 24, T).transpose(0, 2, 3, 1, 4)
    out = np.empty((NQUAD, KS, 4, T), np.float32)
    out[:, :48] = st.reshape(NQUAD, 48, 4, T)
    out[:, 48] = 1.0
    return out.astype(NP_BF16)


_NC_CACHE = {}


def _get_nc():
    if "nc" not in _NC_CACHE:
        _NC_CACHE["nc"] = build_nc()
    return _NC_CACHE["nc"]


def kernel(x, W_in, b_in, W_ih, b_ih, W_hh, b_hh, W_out, b_out, _trace=False):
    from concourse.bass_utils import run_bass_kernel_spmd

    x = np.asarray(x, np.float32)
    l1, l2, l3, b2v, bov = prep_weights(
        W_in, b_in, W_ih, b_ih, b_hh, W_out, b_out, x
    )
    in_maps = []
    for core in range(NCORES):
        in_maps.append(
            {
                "xp": prep_x_core(x, core),
                "l1": l1,
                "l2": l2,
                "l3": l3,
                "b2": b2v,
                "bo": bov,
            }
        )
    nc = _get_nc()
    res = run_bass_kernel_spmd(nc, in_maps, list(range(NCORES)), trace=_trace)
    _NC_CACHE["last_result"] = res

    out = np.empty((T, NG_ALL, 1), np.float32)
    out[:3, :, 0] = x[:3, :, 7]
    for core in range(NCORES):
        yc = res.results[core]["y"]  # [128, T]
        out[3:, core * G_CORE : (core + 1) * G_CORE, 0] = yc[:, 3:].T
    return out
